# revision 7
# baseline (speedup 1.0000x reference)
"""Multi-head attention (B=4, S=2048, D=1024, H=16) on 8 TRN2 NeuronCores.

Sharding: core c handles batch b = c//2 and query-half qh = c%2 (1024 query
rows), with K/V projection for its batch replicated across the 2 cores that
share the batch. Zero inter-core communication; host just slices inputs and
concatenates outputs.

Per-core dataflow (all matmuls float32r unless noted):
  1. PE-transpose inputs to X^T layout ([d_model on partitions, seq free]).
  2. Projections: Q^T/K^T = W^T chunks @ X^T  (evicted to bf16, +bias),
     V = X^T-chunks(stationary) @ Wv (normal [s, dv] layout, f32r).
  3. Per head-pair, per q-tile(512): scores^T = K_h^T.T @ Q_h^T (bf16 matmul,
     2 heads row-packed in the PE array), exp via ScalarE (scale=1/32) to
     f32r, PV col-packed (2 heads), softmax sums via ones-matmul (M=1),
     normalize O^T with GPSIMD-broadcast reciprocals (+bv).
  4. Final: out = O^T-chunks.T @ Wo + bo (bo added via a K=1 ones matmul).
"""

import numpy as np

import concourse.bacc as bacc
import concourse.mybir as mybir
import concourse.tile as tile
from concourse import bass_utils
from concourse.masks import make_identity

F32 = mybir.dt.float32
F32R = mybir.dt.float32r
BF16 = mybir.dt.bfloat16
EXP = mybir.ActivationFunctionType.Exp
COPY = mybir.ActivationFunctionType.Copy

B, S, D, H = 4, 2048, 1024, 16
SQ = 1024          # query rows per core
P = 128
MC = D // P        # 8 m-chunks (contraction of projections)
DKC = D // P       # 8 dk-chunks
KC = S // P        # 16 key chunks
SCALE = 1.0 / 32.0  # 1/sqrt(D_K)
N_CORES = 8

_CACHED_NC = None


def build_nc():
    nc = bacc.Bacc("TRN2", target_bir_lowering=False, debug=False,
                   num_devices=N_CORES)
    q_in = nc.dram_tensor("q_in", [SQ, D], F32, kind="ExternalInput")
    k_in = nc.dram_tensor("k_in", [S, D], F32, kind="ExternalInput")
    v_in = nc.dram_tensor("v_in", [S, D], F32, kind="ExternalInput")
    wq_d = nc.dram_tensor("wq", [D, D], F32, kind="ExternalInput")
    wk_d = nc.dram_tensor("wk", [D, D], F32, kind="ExternalInput")
    wv_d = nc.dram_tensor("wv", [D, D], F32, kind="ExternalInput")
    wo_d = nc.dram_tensor("wo", [D, D], F32, kind="ExternalInput")
    bq_d = nc.dram_tensor("bq", [D], F32, kind="ExternalInput")
    bk_d = nc.dram_tensor("bk", [D], F32, kind="ExternalInput")
    bv_d = nc.dram_tensor("bv", [D], F32, kind="ExternalInput")
    bo_d = nc.dram_tensor("bo", [D], F32, kind="ExternalInput")
    out_d = nc.dram_tensor("out", [SQ, D], F32, kind="ExternalOutput")

    with tile.TileContext(nc) as tc:
        with tc.tile_pool(name="const", bufs=1) as constp:
            ident = constp.tile([P, P], F32)
            make_identity(nc, ident[:])
            ones_f = constp.tile([P, 1], F32)
            nc.vector.memset(ones_f[:], 1.0)
            ones_col = constp.tile([P, 1], F32R)
            nc.vector.tensor_copy(ones_col[:], ones_f[:])
            onesr_f = constp.tile([1, P], F32)
            nc.vector.memset(onesr_f[:], 1.0)
            ones_row = constp.tile([1, P], F32R)
            nc.vector.tensor_copy(ones_row[:], onesr_f[:])
            bq_t = constp.tile([P, MC], F32)
            nc.sync.dma_start(bq_t[:], bq_d.ap().rearrange("(c p) -> p c", p=P))
            bk_t = constp.tile([P, MC], F32)
            nc.sync.dma_start(bk_t[:], bk_d.ap().rearrange("(c p) -> p c", p=P))
            bv_t = constp.tile([P, MC], F32)
            nc.sync.dma_start(bv_t[:], bv_d.ap().rearrange("(c p) -> p c", p=P))
            bo_f = constp.tile([1, D], F32)
            nc.sync.dma_start(bo_f[:], bo_d.ap().unsqueeze(0))
            bo_t = constp.tile([1, D], F32R)
            nc.vector.tensor_copy(bo_t[:], bo_f[:])

            _build_body(nc, tc, q_in, k_in, v_in, wq_d, wk_d, wv_d, wo_d,
                        bq_t, bk_t, bv_t, bo_t, ident, ones_col, ones_row,
                        out_d)
    nc.compile()
    return nc


def _load_w(nc, wpool, stg, w_d, tag):
    """DMA weight matrix row-chunks and round to f32r. Returns 8 tiles
    [128, D] (f32r), tile mm = rows [128*mm, 128*mm+128)."""
    tiles = []
    for mm in range(MC):
        raw = stg.tile([P, D], F32, tag="wraw")
        nc.sync.dma_start(raw[:], w_d.ap()[mm * P:(mm + 1) * P, :])
        t = wpool.tile([P, D], F32R, tag=f"{tag}{mm}", name=f"wt_{tag}{mm}")
        nc.vector.tensor_copy(t[:], raw[:])
        tiles.append(t)
    return tiles


def _transpose_groups(nc, x_d, n_rows, stg, ps_t, ident, evict):
    """PE-transpose x_d [n_rows, D] in groups of 4 row-chunks.

    For each group g and m-chunk mm, produces a [128, 512] transposed block
    (partitions = m, free = the group's 4x128 seq rows) in PSUM and calls
    evict(mm, g, psum_slice) to store it."""
    ngroups = n_rows // (4 * P)
    for g in range(ngroups):
        rows = []
        for j in range(4):
            r = g * 4 + j
            t = stg.tile([P, D], F32, tag="xin", bufs=6)
            nc.sync.dma_start(t[:], x_d.ap()[r * P:(r + 1) * P, :])
            rows.append(t)
        for mm in range(MC):
            pst = ps_t.tile([P, 512], F32, tag="pst")
            for j in range(4):
                nc.tensor.transpose(
                    pst[:, j * P:(j + 1) * P],
                    rows[j][:, mm * P:(mm + 1) * P], ident[:])
            evict(mm, g, pst)


def _build_body(nc, tc, q_in, k_in, v_in, wq_d, wk_d, wv_d, wo_d,
                bq_t, bk_t, bv_t, bo_t, ident, ones_col, ones_row, out_d):
    # ---------------- persistent pools (LIFO stack) ----------------
    with tc.tile_pool(name="qtp", bufs=1) as qtp:
        QT = [qtp.tile([P, SQ], BF16, tag=f"qt{i}", name=f"qt{i}") for i in range(DKC)]

        # ---- stage Q ----
        with (
            tc.tile_pool(name="xtq", bufs=1) as xtp,
            tc.tile_pool(name="wq", bufs=1) as wpool,
            tc.tile_pool(name="stgq", bufs=2) as stg,
            tc.tile_pool(name="psq_t", bufs=2, space="PSUM") as ps_t,
            tc.tile_pool(name="psq_p", bufs=2, space="PSUM") as ps_p,
        ):
            xqT = [xtp.tile([P, SQ], F32R, tag=f"xt{i}", name=f"xqt{i}") for i in range(MC)]
            wq_t = _load_w(nc, wpool, stg, wq_d, "w")

            def evq(mm, g, pst):
                nc.scalar.activation(
                    xqT[mm][:, g * 512:(g + 1) * 512], pst[:], COPY)
            _transpose_groups(nc, q_in, SQ, stg, ps_t, ident, evq)

            for dk in range(DKC):
                for nh in range(SQ // 512):
                    ps = ps_p.tile([P, 512], F32, tag="pp")
                    for mm in range(MC):
                        nc.tensor.matmul(
                            ps[:], wq_t[mm][:, dk * P:(dk + 1) * P],
                            xqT[mm][:, nh * 512:(nh + 1) * 512],
                            start=(mm == 0), stop=(mm == MC - 1))
                    nc.vector.tensor_scalar_add(
                        QT[dk][:, nh * 512:(nh + 1) * 512], ps[:],
                        bq_t[:, dk:dk + 1])

        with tc.tile_pool(name="ktp", bufs=1) as ktp:
            KT = [ktp.tile([P, S], BF16, tag=f"kt{i}", name=f"kt{i}") for i in range(DKC)]

            # ---- stage K ----
            with (
                tc.tile_pool(name="xtk", bufs=1) as xtp,
                tc.tile_pool(name="wk", bufs=1) as wpool,
                tc.tile_pool(name="stgk", bufs=2) as stg,
                tc.tile_pool(name="psk_t", bufs=2, space="PSUM") as ps_t,
                tc.tile_pool(name="psk_p", bufs=2, space="PSUM") as ps_p,
            ):
                xkT = [xtp.tile([P, S], F32R, tag=f"xt{i}", name=f"xkt{i}") for i in range(MC)]
                wk_t = _load_w(nc, wpool, stg, wk_d, "w")

                def evk(mm, g, pst):
                    nc.scalar.activation(
                        xkT[mm][:, g * 512:(g + 1) * 512], pst[:], COPY)
                _transpose_groups(nc, k_in, S, stg, ps_t, ident, evk)

                for dk in range(DKC):
                    for nh in range(S // 512):
                        ps = ps_p.tile([P, 512], F32, tag="pp")
                        for mm in range(MC):
                            nc.tensor.matmul(
                                ps[:], wk_t[mm][:, dk * P:(dk + 1) * P],
                                xkT[mm][:, nh * 512:(nh + 1) * 512],
                                start=(mm == 0), stop=(mm == MC - 1))
                        nc.vector.tensor_scalar_add(
                            KT[dk][:, nh * 512:(nh + 1) * 512], ps[:],
                            bk_t[:, dk:dk + 1])

            with tc.tile_pool(name="vp", bufs=1) as vp:
                DEXT = H * 65  # V_ext: 65 cols per head (64 V + ones)
                V = [vp.tile([P, DEXT], F32R, tag=f"v{i}", name=f"v{i}")
                     for i in range(KC)]

                # ---- stage V ----
                with (
                    tc.tile_pool(name="vtt", bufs=1) as vtt,
                    tc.tile_pool(name="wv", bufs=1) as wpool,
                    tc.tile_pool(name="stgv", bufs=2) as stg,
                    tc.tile_pool(name="psv_t", bufs=2, space="PSUM") as ps_t,
                    tc.tile_pool(name="psv_p", bufs=2, space="PSUM") as ps_p,
                ):
                    wv_t = _load_w(nc, wpool, stg, wv_d, "w")
                    valT = [vtt.tile([P, 512], F32R, tag=f"vt{i}", name=f"vt{i}")
                            for i in range(MC)]
                    ones16 = vtt.tile([P, H], F32, name="ones16")
                    nc.vector.memset(ones16[:], 1.0)

                    ngroups = S // (4 * P)
                    for g in range(ngroups):
                        rows = []
                        for j in range(4):
                            r = g * 4 + j
                            t = stg.tile([P, D], F32, tag="xin", bufs=6)
                            nc.sync.dma_start(t[:], v_in.ap()[r * P:(r + 1) * P, :])
                            rows.append(t)
                        for mm in range(MC):
                            pst = ps_t.tile([P, 512], F32, tag="pst")
                            for j in range(4):
                                nc.tensor.transpose(
                                    pst[:, j * P:(j + 1) * P],
                                    rows[j][:, mm * P:(mm + 1) * P], ident[:])
                            nc.scalar.activation(valT[mm][:], pst[:], COPY)
                        for j in range(4):
                            sc = g * 4 + j
                            vx = V[sc].rearrange("p (h c) -> p h c", c=65)
                            nc.vector.tensor_copy(
                                vx[:, :, 64:65],
                                ones16[:].rearrange("p (h c) -> p h c", c=1))
                            for nh in range(2):
                                ps = ps_p.tile([P, 512], F32, tag="pp")
                                for mm in range(MC):
                                    nc.tensor.matmul(
                                        ps[:], valT[mm][:, j * P:(j + 1) * P],
                                        wv_t[mm][:, nh * 512:(nh + 1) * 512],
                                        start=(mm == 0), stop=(mm == MC - 1))
                                nc.vector.tensor_copy(
                                    vx[:, 8 * nh:8 * nh + 8, 0:64],
                                    ps[:].rearrange("p (h c) -> p h c", c=64))

                with tc.tile_pool(name="otp", bufs=1) as otp:
                    OT = [otp.tile([P, SQ], F32R, tag=f"ot{i}", name=f"ot{i}")
                          for i in range(DKC)]

                    # ---- attention + final ----
                    with (
                        tc.tile_pool(name="ep", bufs=6) as ep,
                        tc.tile_pool(name="bcp", bufs=2) as bcp,
                        tc.tile_pool(name="rp", bufs=2) as rp,
                        tc.tile_pool(name="ps_sc", bufs=4, space="PSUM") as ps_sc,
                        tc.tile_pool(name="ps_pv", bufs=3, space="PSUM") as ps_pv,
                        tc.tile_pool(name="wo", bufs=1) as wop,
                        tc.tile_pool(name="fin", bufs=2) as finp,
                        tc.tile_pool(name="ps_f", bufs=1, space="PSUM") as ps_f,
                    ):
                        for qt in range(SQ // 512):
                            qs = slice(qt * 512, (qt + 1) * 512)
                            for pair in range(H // 2):
                                pv1 = ps_pv.tile([P, 512], F32, tag="pv")
                                pv2 = ps_pv.tile([P, 512], F32, tag="pv")
                                for kc in range(KC):
                                    ks = slice(kc * P, (kc + 1) * P)
                                    first, last = kc == 0, kc == KC - 1
                                    s1 = ps_sc.tile([P, 512], F32, tag="sc")
                                    s2 = ps_sc.tile([P, 512], F32, tag="sc")
                                    nc.tensor.matmul(
                                        s1[:], KT[pair][0:64, ks],
                                        QT[pair][0:64, qs],
                                        start=True, stop=True,
                                        tile_position=(0, 0))
                                    nc.tensor.matmul(
                                        s2[:], KT[pair][64:128, ks],
                                        QT[pair][64:128, qs],
                                        start=True, stop=True,
                                        tile_position=(64, 0))
                                    e1 = ep.tile([P, 512], F32R, tag="e")
                                    e2 = ep.tile([P, 512], F32R, tag="e")
                                    nc.scalar.activation(e1[:], s1[:], EXP,
                                                         scale=SCALE)
                                    nc.scalar.activation(e2[:], s2[:], EXP,
                                                         scale=SCALE)
                                    c1 = (2 * pair) * 65
                                    c2 = (2 * pair + 1) * 65
                                    nc.tensor.matmul(
                                        pv1[0:65, :], V[kc][:, c1:c1 + 65],
                                        e1[:], start=first, stop=last)
                                    nc.tensor.matmul(
                                        pv2[0:65, :], V[kc][:, c2:c2 + 65],
                                        e2[:], start=first, stop=last)
                                # normalize both heads; odd head DMA-shifted
                                for hh, pvp in ((0, pv1), (1, pv2)):
                                    rb = rp.tile([P, 512], F32, tag="rb")
                                    nc.vector.reciprocal(rb[64:65, :],
                                                         pvp[64:65, :])
                                    r0 = rp.tile([1, 512], F32, tag="r0")
                                    nc.gpsimd.tensor_copy(r0[:], rb[64:65, :])
                                    bc = bcp.tile([64, 512], F32, tag="bc")
                                    nc.gpsimd.partition_broadcast(bc[:], r0[:])
                                    if hh == 0:
                                        osl = OT[pair][0:64, qs]
                                        nc.vector.tensor_mul(
                                            osl, pvp[0:64, :], bc[:])
                                        nc.vector.tensor_scalar_add(
                                            osl, osl, bv_t[0:64, pair:pair + 1])
                                    else:
                                        tmp = bcp.tile([64, 512], F32R,
                                                       tag="tmp")
                                        nc.vector.tensor_mul(
                                            tmp[:], pvp[0:64, :], bc[:])
                                        osl = OT[pair][64:128, qs]
                                        nc.sync.dma_start(osl, tmp[:])
                                        nc.vector.tensor_scalar_add(
                                            osl, osl,
                                            bv_t[64:128, pair:pair + 1])

                            # final projection for this q-tile's s-chunks
                            for nh in range(2):
                                ns = slice(nh * 512, (nh + 1) * 512)
                                wo_h = []
                                for dk in range(DKC):
                                    raw = finp.tile([P, 512], F32, tag="wraw")
                                    nc.sync.dma_start(
                                        raw[:], wo_d.ap()[dk * P:(dk + 1) * P, ns])
                                    wt = wop.tile([P, 512], F32R,
                                                  tag=f"woh{dk}", name=f"woh{dk}")
                                    nc.vector.tensor_copy(wt[:], raw[:])
                                    wo_h.append(wt)
                                for sc in range(qt * 4, (qt + 1) * 4):
                                    ss = slice(sc * P, (sc + 1) * P)
                                    fps = ps_f.tile([P, 512], F32, tag="f")
                                    for dk in range(DKC):
                                        nc.tensor.matmul(
                                            fps[:], OT[dk][:, ss],
                                            wo_h[dk][:],
                                            start=(dk == 0), stop=False)
                                    nc.tensor.matmul(
                                        fps[:], ones_row[:], bo_t[:, ns],
                                        start=False, stop=True)
                                    ob = finp.tile([P, 512], F32, tag="ob")
                                    nc.vector.tensor_copy(ob[:], fps[:])
                                    nc.sync.dma_start(out_d.ap()[ss, ns], ob[:])


def get_nc():
    global _CACHED_NC
    if _CACHED_NC is None:
        _CACHED_NC = build_nc()
    return _CACHED_NC


def run(inputs, **kwargs):
    """Run on 8 cores; returns (full_output, BassKernelResults)."""
    nc = get_nc()
    queries = np.ascontiguousarray(np.asarray(inputs["queries"], np.float32))
    keys = np.ascontiguousarray(np.asarray(inputs["keys"], np.float32))
    values = np.ascontiguousarray(np.asarray(inputs["values"], np.float32))
    base = {
        "wq": np.ascontiguousarray(np.asarray(inputs["Wq"], np.float32)),
        "wk": np.ascontiguousarray(np.asarray(inputs["Wk"], np.float32)),
        "wv": np.ascontiguousarray(np.asarray(inputs["Wv"], np.float32)),
        "wo": np.ascontiguousarray(np.asarray(inputs["Wo"], np.float32)),
        "bq": np.ascontiguousarray(np.asarray(inputs["bq"], np.float32)),
        "bk": np.ascontiguousarray(np.asarray(inputs["bk"], np.float32)),
        "bv": np.ascontiguousarray(np.asarray(inputs["bv"], np.float32)),
        "bo": np.ascontiguousarray(np.asarray(inputs["bo"], np.float32)),
    }
    in_maps = []
    for c in range(N_CORES):
        b, qh = c // 2, c % 2
        m = dict(base)
        m["q_in"] = np.ascontiguousarray(queries[b, qh * SQ:(qh + 1) * SQ])
        m["k_in"] = keys[b]
        m["v_in"] = values[b]
        in_maps.append(m)
    res = bass_utils.run_bass_kernel_spmd(
        nc, in_maps, core_ids=list(range(N_CORES)), **kwargs)
    out = np.empty((B, S, D), np.float32)
    for c in range(N_CORES):
        b, qh = c // 2, c % 2
        out[b, qh * SQ:(qh + 1) * SQ] = res.results[c]["out"]
    return out, res


def kernel(**inputs):
    out, _ = run(inputs)
    return out


if __name__ == "__main__":
    rng = np.random.default_rng(0)
    ins = {
        "queries": rng.standard_normal((B, S, D), dtype=np.float32),
        "keys": rng.standard_normal((B, S, D), dtype=np.float32),
        "values": rng.standard_normal((B, S, D), dtype=np.float32),
        "Wq": (rng.standard_normal((D, D), dtype=np.float32) / 32),
        "bq": np.zeros(D, np.float32),
        "Wk": (rng.standard_normal((D, D), dtype=np.float32) / 32),
        "bk": np.zeros(D, np.float32),
        "Wv": (rng.standard_normal((D, D), dtype=np.float32) / 32),
        "bv": np.zeros(D, np.float32),
        "Wo": (rng.standard_normal((D, D), dtype=np.float32) / 32),
        "bo": np.zeros(D, np.float32),
    }
    out = kernel(**ins)
    print("out", out.shape, out.dtype, np.abs(out).mean())


# revision 12
# speedup vs baseline: 1.2389x; 1.2389x over previous
"""Multi-head attention (B=4, S=2048, D=1024, H=16) on 8 TRN2 NeuronCores.

Sharding: core c handles batch b = c//2 and query-half qh = c%2 (1024 query
rows), with K/V projection for its batch replicated across the 2 cores that
share the batch. Zero inter-core communication; host just slices inputs and
concatenates outputs.

Per-core dataflow (all matmuls float32r unless noted):
  1. PE-transpose inputs to X^T layout ([d_model on partitions, seq free]).
  2. Projections: Q^T/K^T = W^T chunks @ X^T  (evicted to bf16, +bias),
     V = X^T-chunks(stationary) @ Wv (normal [s, dv] layout, f32r).
  3. Per head-pair, per q-tile(512): scores^T = K_h^T.T @ Q_h^T (bf16 matmul,
     2 heads row-packed in the PE array), exp via ScalarE (scale=1/32) to
     f32r, PV col-packed (2 heads), softmax sums via ones-matmul (M=1),
     normalize O^T with GPSIMD-broadcast reciprocals (+bv).
  4. Final: out = O^T-chunks.T @ Wo + bo (bo added via a K=1 ones matmul).
"""

import numpy as np

import concourse.bacc as bacc
import concourse.mybir as mybir
import concourse.tile as tile
from concourse import bass_utils
from concourse.masks import make_identity

F32 = mybir.dt.float32
F32R = mybir.dt.float32r
BF16 = mybir.dt.bfloat16
EXP = mybir.ActivationFunctionType.Exp
COPY = mybir.ActivationFunctionType.Copy

B, S, D, H = 4, 2048, 1024, 16
SQ = 1024          # query rows per core
P = 128
MC = D // P        # 8 m-chunks (contraction of projections)
DKC = D // P       # 8 dk-chunks
KC = S // P        # 16 key chunks
SCALE = 1.0 / 32.0  # 1/sqrt(D_K)
N_CORES = 8

_CACHED_NC = None


def build_nc():
    nc = bacc.Bacc("TRN2", target_bir_lowering=False, debug=False,
                   num_devices=N_CORES)
    q_in = nc.dram_tensor("q_in", [SQ, D], F32, kind="ExternalInput")
    k_in = nc.dram_tensor("k_in", [S, D], F32, kind="ExternalInput")
    v_in = nc.dram_tensor("v_in", [S, D], F32, kind="ExternalInput")
    wq_d = nc.dram_tensor("wq", [D, D], F32, kind="ExternalInput")
    wk_d = nc.dram_tensor("wk", [D, D], F32, kind="ExternalInput")
    wv_d = nc.dram_tensor("wv", [D, D], F32, kind="ExternalInput")
    wo_d = nc.dram_tensor("wo", [D, D], F32, kind="ExternalInput")
    bq_d = nc.dram_tensor("bq", [D], F32, kind="ExternalInput")
    bk_d = nc.dram_tensor("bk", [D], F32, kind="ExternalInput")
    bv_d = nc.dram_tensor("bv", [D], F32, kind="ExternalInput")
    bo_d = nc.dram_tensor("bo", [D], F32, kind="ExternalInput")
    out_d = nc.dram_tensor("out", [SQ, D], F32, kind="ExternalOutput")

    with tile.TileContext(nc) as tc:
        with tc.tile_pool(name="const", bufs=1) as constp:
            ident = constp.tile([P, P], F32)
            make_identity(nc, ident[:])
            ones_f = constp.tile([P, 1], F32)
            nc.vector.memset(ones_f[:], 1.0)
            ones_col = constp.tile([P, 1], F32R)
            nc.vector.tensor_copy(ones_col[:], ones_f[:])
            onesr_f = constp.tile([1, P], F32)
            nc.vector.memset(onesr_f[:], 1.0)
            ones_row = constp.tile([1, P], F32R)
            nc.vector.tensor_copy(ones_row[:], onesr_f[:])
            bq_t = constp.tile([P, MC], F32)
            nc.sync.dma_start(bq_t[:], bq_d.ap().rearrange("(c p) -> p c", p=P))
            bk_t = constp.tile([P, MC], F32)
            nc.sync.dma_start(bk_t[:], bk_d.ap().rearrange("(c p) -> p c", p=P))
            bv_t = constp.tile([P, MC], F32)
            nc.sync.dma_start(bv_t[:], bv_d.ap().rearrange("(c p) -> p c", p=P))
            bo_f = constp.tile([1, D], F32)
            nc.sync.dma_start(bo_f[:], bo_d.ap().unsqueeze(0))
            bo_t = constp.tile([1, D], F32R)
            nc.vector.tensor_copy(bo_t[:], bo_f[:])

            _build_body(nc, tc, q_in, k_in, v_in, wq_d, wk_d, wv_d, wo_d,
                        bq_t, bk_t, bv_t, bo_t, ident, ones_col, ones_row,
                        out_d)
    nc.compile()
    return nc


def _load_w(nc, wpool, stg, w_d, tag):
    """DMA weight matrix row-chunks and round to f32r. Returns 8 tiles
    [128, D] (f32r), tile mm = rows [128*mm, 128*mm+128)."""
    tiles = []
    for mm in range(MC):
        raw = stg.tile([P, D], F32, tag="wraw")
        nc.sync.dma_start(raw[:], w_d.ap()[mm * P:(mm + 1) * P, :])
        t = wpool.tile([P, D], F32R, tag=f"{tag}{mm}", name=f"wt_{tag}{mm}")
        nc.vector.tensor_copy(t[:], raw[:])
        tiles.append(t)
    return tiles


def _transpose_groups(nc, x_d, n_rows, stg, ps_t, ident, evict):
    """PE-transpose x_d [n_rows, D] in groups of 4 row-chunks.

    For each group g and m-chunk mm, produces a [128, 512] transposed block
    (partitions = m, free = the group's 4x128 seq rows) in PSUM and calls
    evict(mm, g, psum_slice) to store it."""
    ngroups = n_rows // (4 * P)
    for g in range(ngroups):
        rows = []
        for j in range(4):
            r = g * 4 + j
            t = stg.tile([P, D], F32, tag="xin", bufs=6)
            nc.sync.dma_start(t[:], x_d.ap()[r * P:(r + 1) * P, :])
            rows.append(t)
        for mm in range(MC):
            pst = ps_t.tile([P, 512], F32, tag="pst")
            for j in range(4):
                nc.tensor.transpose(
                    pst[:, j * P:(j + 1) * P],
                    rows[j][:, mm * P:(mm + 1) * P], ident[:])
            evict(mm, g, pst)


def _build_body(nc, tc, q_in, k_in, v_in, wq_d, wk_d, wv_d, wo_d,
                bq_t, bk_t, bv_t, bo_t, ident, ones_col, ones_row, out_d):
    # ---------------- persistent pools (LIFO stack) ----------------
    with tc.tile_pool(name="qtp", bufs=1) as qtp:
        QT = [qtp.tile([P, SQ], BF16, tag=f"qt{i}", name=f"qt{i}") for i in range(DKC)]

        # ---- stage Q ----
        with (
            tc.tile_pool(name="xtq", bufs=1) as xtp,
            tc.tile_pool(name="wq", bufs=1) as wpool,
            tc.tile_pool(name="stgq", bufs=2) as stg,
            tc.tile_pool(name="psq_t", bufs=2, space="PSUM") as ps_t,
            tc.tile_pool(name="psq_p", bufs=2, space="PSUM") as ps_p,
        ):
            xqT = [xtp.tile([P, SQ], F32R, tag=f"xt{i}", name=f"xqt{i}") for i in range(MC)]
            wq_t = _load_w(nc, wpool, stg, wq_d, "w")

            def evq(mm, g, pst):
                nc.scalar.activation(
                    xqT[mm][:, g * 512:(g + 1) * 512], pst[:], COPY)
            _transpose_groups(nc, q_in, SQ, stg, ps_t, ident, evq)

            for dk in range(DKC):
                for nh in range(SQ // 512):
                    ps = ps_p.tile([P, 512], F32, tag="pp")
                    for mm in range(MC):
                        nc.tensor.matmul(
                            ps[:], wq_t[mm][:, dk * P:(dk + 1) * P],
                            xqT[mm][:, nh * 512:(nh + 1) * 512],
                            start=(mm == 0), stop=(mm == MC - 1))
                    nc.vector.tensor_scalar_add(
                        QT[dk][:, nh * 512:(nh + 1) * 512], ps[:],
                        bq_t[:, dk:dk + 1])

        with tc.tile_pool(name="ktp", bufs=1) as ktp:
            KT = [ktp.tile([P, S], BF16, tag=f"kt{i}", name=f"kt{i}") for i in range(DKC)]

            # ---- stage K ----
            with (
                tc.tile_pool(name="xtk", bufs=1) as xtp,
                tc.tile_pool(name="wk", bufs=1) as wpool,
                tc.tile_pool(name="stgk", bufs=2) as stg,
                tc.tile_pool(name="psk_t", bufs=2, space="PSUM") as ps_t,
                tc.tile_pool(name="psk_p", bufs=2, space="PSUM") as ps_p,
            ):
                xkT = [xtp.tile([P, S], F32R, tag=f"xt{i}", name=f"xkt{i}") for i in range(MC)]
                wk_t = _load_w(nc, wpool, stg, wk_d, "w")

                def evk(mm, g, pst):
                    nc.scalar.activation(
                        xkT[mm][:, g * 512:(g + 1) * 512], pst[:], COPY)
                _transpose_groups(nc, k_in, S, stg, ps_t, ident, evk)

                for dk in range(DKC):
                    for nh in range(S // 512):
                        ps = ps_p.tile([P, 512], F32, tag="pp")
                        for mm in range(MC):
                            nc.tensor.matmul(
                                ps[:], wk_t[mm][:, dk * P:(dk + 1) * P],
                                xkT[mm][:, nh * 512:(nh + 1) * 512],
                                start=(mm == 0), stop=(mm == MC - 1))
                        nc.vector.tensor_scalar_add(
                            KT[dk][:, nh * 512:(nh + 1) * 512], ps[:],
                            bk_t[:, dk:dk + 1])

            with tc.tile_pool(name="vp", bufs=1) as vp:
                DEXT = H * 65  # V_ext: 65 cols per head (64 V + ones)
                V = [vp.tile([P, DEXT], F32R, tag=f"v{i}", name=f"v{i}")
                     for i in range(KC)]

                # ---- stage V ----
                with (
                    tc.tile_pool(name="vtt", bufs=1) as vtt,
                    tc.tile_pool(name="wv", bufs=1) as wpool,
                    tc.tile_pool(name="stgv", bufs=2) as stg,
                    tc.tile_pool(name="psv_t", bufs=2, space="PSUM") as ps_t,
                    tc.tile_pool(name="psv_p", bufs=2, space="PSUM") as ps_p,
                ):
                    wv_t = _load_w(nc, wpool, stg, wv_d, "w")
                    valT = [vtt.tile([P, 512], F32R, tag=f"vt{i}", name=f"vt{i}")
                            for i in range(MC)]
                    ones16 = vtt.tile([P, H], F32, name="ones16")
                    nc.vector.memset(ones16[:], 1.0)

                    ngroups = S // (4 * P)
                    for g in range(ngroups):
                        rows = []
                        for j in range(4):
                            r = g * 4 + j
                            t = stg.tile([P, D], F32, tag="xin", bufs=6)
                            nc.sync.dma_start(t[:], v_in.ap()[r * P:(r + 1) * P, :])
                            rows.append(t)
                        for mm in range(MC):
                            pst = ps_t.tile([P, 512], F32, tag="pst")
                            for j in range(4):
                                nc.tensor.transpose(
                                    pst[:, j * P:(j + 1) * P],
                                    rows[j][:, mm * P:(mm + 1) * P], ident[:])
                            nc.scalar.activation(valT[mm][:], pst[:], COPY)
                        for j in range(4):
                            sc = g * 4 + j
                            vx = V[sc].rearrange("p (h c) -> p h c", c=65)
                            nc.vector.tensor_copy(
                                vx[:, :, 64:65],
                                ones16[:].rearrange("p (h c) -> p h c", c=1))
                            for nh in range(2):
                                ps = ps_p.tile([P, 512], F32, tag="pp")
                                for mm in range(MC):
                                    nc.tensor.matmul(
                                        ps[:], valT[mm][:, j * P:(j + 1) * P],
                                        wv_t[mm][:, nh * 512:(nh + 1) * 512],
                                        start=(mm == 0), stop=(mm == MC - 1))
                                nc.vector.tensor_copy(
                                    vx[:, 8 * nh:8 * nh + 8, 0:64],
                                    ps[:].rearrange("p (h c) -> p h c", c=64))

                with tc.tile_pool(name="otp", bufs=1) as otp:
                    OT = [otp.tile([P, SQ], F32R, tag=f"ot{i}", name=f"ot{i}")
                          for i in range(DKC)]

                    # ---- attention + final ----
                    with (
                        tc.tile_pool(name="ep", bufs=3) as ep,
                        tc.tile_pool(name="bcp", bufs=2) as bcp,
                        tc.tile_pool(name="rp", bufs=3) as rp,
                        tc.tile_pool(name="ps_sc", bufs=2, space="PSUM") as ps_sc,
                        tc.tile_pool(name="ps_pv", bufs=3, space="PSUM") as ps_pv,
                        tc.tile_pool(name="wo", bufs=1) as wop,
                        tc.tile_pool(name="fin", bufs=2) as finp,
                        tc.tile_pool(name="ps_f", bufs=1, space="PSUM") as ps_f,
                    ):
                        for qt in range(SQ // 512):
                            qs = slice(qt * 512, (qt + 1) * 512)
                            for pair in range(H // 2):
                                pv1 = ps_pv.tile([P, 512], F32, tag="pv")
                                pv2 = ps_pv.tile([P, 512], F32, tag="pv")
                                c1 = (2 * pair) * 65
                                c2 = (2 * pair + 1) * 65
                                for k2 in range(KC // 2):
                                    ka = slice(2 * k2 * P, (2 * k2 + 1) * P)
                                    kb = slice((2 * k2 + 1) * P,
                                               (2 * k2 + 2) * P)
                                    s1 = ps_sc.tile([P, 1024], F32, tag="sc")
                                    s2 = ps_sc.tile([P, 1024], F32, tag="sc")
                                    nc.tensor.matmul(
                                        s1[:, 0:512], KT[pair][0:64, ka],
                                        QT[pair][0:64, qs],
                                        start=True, stop=True,
                                        tile_position=(0, 0))
                                    nc.tensor.matmul(
                                        s2[:, 0:512], KT[pair][64:128, ka],
                                        QT[pair][64:128, qs],
                                        start=True, stop=True,
                                        tile_position=(64, 0))
                                    nc.tensor.matmul(
                                        s1[:, 512:1024], KT[pair][0:64, kb],
                                        QT[pair][0:64, qs],
                                        start=True, stop=True,
                                        tile_position=(0, 0))
                                    nc.tensor.matmul(
                                        s2[:, 512:1024], KT[pair][64:128, kb],
                                        QT[pair][64:128, qs],
                                        start=True, stop=True,
                                        tile_position=(64, 0))
                                    e1 = ep.tile([P, 1024], F32R, tag="e")
                                    e2 = ep.tile([P, 1024], F32R, tag="e")
                                    nc.scalar.activation(e1[:], s1[:], EXP,
                                                         scale=SCALE)
                                    nc.scalar.activation(e2[:], s2[:], EXP,
                                                         scale=SCALE)
                                    first, last = k2 == 0, k2 == KC // 2 - 1
                                    nc.tensor.matmul(
                                        pv1[0:65, :],
                                        V[2 * k2][:, c1:c1 + 65],
                                        e1[:, 0:512], start=first, stop=False)
                                    nc.tensor.matmul(
                                        pv2[0:65, :],
                                        V[2 * k2][:, c2:c2 + 65],
                                        e2[:, 0:512], start=first, stop=False)
                                    nc.tensor.matmul(
                                        pv1[0:65, :],
                                        V[2 * k2 + 1][:, c1:c1 + 65],
                                        e1[:, 512:1024], start=False,
                                        stop=last)
                                    nc.tensor.matmul(
                                        pv2[0:65, :],
                                        V[2 * k2 + 1][:, c2:c2 + 65],
                                        e2[:, 512:1024], start=False,
                                        stop=last)
                                # normalize both heads; odd head DMA-shifted
                                for hh, pvp in ((0, pv1), (1, pv2)):
                                    rb = rp.tile([P, 512], F32, tag="rb", bufs=1)
                                    nc.vector.tensor_copy(rb[64:65, :],
                                                          pvp[64:65, :])
                                    r0 = rp.tile([1, 512], F32, tag="r0")
                                    nc.gpsimd.tensor_copy(r0[:], rb[64:65, :])
                                    rr = rp.tile([1, 512], F32, tag="rr")
                                    nc.vector.reciprocal_approx_fast(rr[:],
                                                                     r0[:])
                                    bc = bcp.tile([64, 512], F32, tag="bc")
                                    nc.gpsimd.partition_broadcast(bc[:], rr[:])
                                    if hh == 0:
                                        osl = OT[pair][0:64, qs]
                                        nc.vector.tensor_mul(
                                            osl, pvp[0:64, :], bc[:])
                                        nc.vector.tensor_scalar_add(
                                            osl, osl, bv_t[0:64, pair:pair + 1])
                                    else:
                                        tmp = bcp.tile([64, 512], F32R,
                                                       tag="tmp", bufs=1)
                                        nc.vector.tensor_mul(
                                            tmp[:], pvp[0:64, :], bc[:])
                                        osl = OT[pair][64:128, qs]
                                        nc.sync.dma_start(osl, tmp[:])
                                        nc.vector.tensor_scalar_add(
                                            osl, osl,
                                            bv_t[64:128, pair:pair + 1])

                            # final projection for this q-tile's s-chunks
                            for nh in range(2):
                                ns = slice(nh * 512, (nh + 1) * 512)
                                wo_h = []
                                for dk in range(DKC):
                                    raw = finp.tile([P, 512], F32, tag="wraw", bufs=1)
                                    nc.sync.dma_start(
                                        raw[:], wo_d.ap()[dk * P:(dk + 1) * P, ns])
                                    wt = wop.tile([P, 512], F32R,
                                                  tag=f"woh{dk}", name=f"woh{dk}")
                                    nc.vector.tensor_copy(wt[:], raw[:])
                                    wo_h.append(wt)
                                for sc in range(qt * 4, (qt + 1) * 4):
                                    ss = slice(sc * P, (sc + 1) * P)
                                    fps = ps_f.tile([P, 512], F32, tag="f")
                                    for dk in range(DKC):
                                        nc.tensor.matmul(
                                            fps[:], OT[dk][:, ss],
                                            wo_h[dk][:],
                                            start=(dk == 0), stop=False)
                                    nc.tensor.matmul(
                                        fps[:], ones_row[:], bo_t[:, ns],
                                        start=False, stop=True)
                                    ob = finp.tile([P, 512], F32, tag="ob", bufs=1)
                                    nc.vector.tensor_copy(ob[:], fps[:])
                                    nc.sync.dma_start(out_d.ap()[ss, ns], ob[:])


def get_nc():
    global _CACHED_NC
    if _CACHED_NC is None:
        _CACHED_NC = build_nc()
    return _CACHED_NC


def run(inputs, **kwargs):
    """Run on 8 cores; returns (full_output, BassKernelResults)."""
    nc = get_nc()
    queries = np.ascontiguousarray(np.asarray(inputs["queries"], np.float32))
    keys = np.ascontiguousarray(np.asarray(inputs["keys"], np.float32))
    values = np.ascontiguousarray(np.asarray(inputs["values"], np.float32))
    base = {
        "wq": np.ascontiguousarray(np.asarray(inputs["Wq"], np.float32)),
        "wk": np.ascontiguousarray(np.asarray(inputs["Wk"], np.float32)),
        "wv": np.ascontiguousarray(np.asarray(inputs["Wv"], np.float32)),
        "wo": np.ascontiguousarray(np.asarray(inputs["Wo"], np.float32)),
        "bq": np.ascontiguousarray(np.asarray(inputs["bq"], np.float32)),
        "bk": np.ascontiguousarray(np.asarray(inputs["bk"], np.float32)),
        "bv": np.ascontiguousarray(np.asarray(inputs["bv"], np.float32)),
        "bo": np.ascontiguousarray(np.asarray(inputs["bo"], np.float32)),
    }
    in_maps = []
    for c in range(N_CORES):
        b, qh = c // 2, c % 2
        m = dict(base)
        m["q_in"] = np.ascontiguousarray(queries[b, qh * SQ:(qh + 1) * SQ])
        m["k_in"] = keys[b]
        m["v_in"] = values[b]
        in_maps.append(m)
    res = bass_utils.run_bass_kernel_spmd(
        nc, in_maps, core_ids=list(range(N_CORES)), **kwargs)
    out = np.empty((B, S, D), np.float32)
    for c in range(N_CORES):
        b, qh = c // 2, c % 2
        out[b, qh * SQ:(qh + 1) * SQ] = res.results[c]["out"]
    return out, res


def kernel(**inputs):
    out, _ = run(inputs)
    return out


if __name__ == "__main__":
    rng = np.random.default_rng(0)
    ins = {
        "queries": rng.standard_normal((B, S, D), dtype=np.float32),
        "keys": rng.standard_normal((B, S, D), dtype=np.float32),
        "values": rng.standard_normal((B, S, D), dtype=np.float32),
        "Wq": (rng.standard_normal((D, D), dtype=np.float32) / 32),
        "bq": np.zeros(D, np.float32),
        "Wk": (rng.standard_normal((D, D), dtype=np.float32) / 32),
        "bk": np.zeros(D, np.float32),
        "Wv": (rng.standard_normal((D, D), dtype=np.float32) / 32),
        "bv": np.zeros(D, np.float32),
        "Wo": (rng.standard_normal((D, D), dtype=np.float32) / 32),
        "bo": np.zeros(D, np.float32),
    }
    out = kernel(**ins)
    print("out", out.shape, out.dtype, np.abs(out).mean())
